# revision 4
# baseline (speedup 1.0000x reference)
"""Trainium2 Bass kernel for nn_CLUB_816043786555 (CLUB loss).

Full-input contract: kernel(**inputs) takes the complete arrays, shards the
batch dim across 8 NeuronCores, runs a Bass/Tile kernel per core, and
combines tiny per-core partial sums on the host.

Math: with mu = leaky(x@W1m+b1m)@W2m+b2m, logvar = tanh(leaky(x@W1v+b1v)@W2v+b2v),
iv = exp(-logvar), and HOST-precomputed centered targets
  yc  = y - mean_i(y)          [per column d]
  y2c = y^2 - mean_i(y^2)
the loss reduces to (mu^2 cancels analytically):

  loss = -0.5/N * sum_{i,d} iv * (y2c - 2*mu*yc)
       = -0.5/N * (P1 - 2*P2),   P1 = sum iv*y2c,  P2 = sum q*yc,
                                 q  = (mu_psum + b2m)*iv

Host centering eliminates the B/C/S/T accumulators of the uncentered
expansion: the device emits only P1/P2 per 1024-row group (acc[128, 32]).

Schedule: 16 groups of RG=1024 rows, unit period == PE busy time (6827ns =
32 matmuls x 512 cols x 0.417ns).  KEY ORDER CHANGE vs the naive interleave:
each unit runs the L2 slots of the PREVIOUS group FIRST in each quarter
(z-head slots 0-1, mu-head slots 2-3), then the L1 chunk.  That spaces the 8
L1 PSUM writes ~1700ns apart so a plain 2-buffer hp ring gives every leaky
~3400ns of slack - no PSUM-ring stalls, no zps borrow.  tanh(g-1) releases
after slot 1 (~2.7us), exp(g-1) runs the SAME unit, so iv(g-1) is ready a
full unit before q(g-1).

Engine budget per unit (PE 6827 = pacer):
  PE  : 4 L2 slots (4 mm each) + 4 L1 chunks (4 mm each), fp16, 512-col
  ACT : 4x Prelu (bias fused) + tanh + exp                      ~6.4us
  DVE : 4x custom fused bias+leaky (PSUM) + q (scalar_tensor_tensor,
        2 col-pieces 650/374 so mups frees before slot 2 writes) +
        P1/P2 sums (4x-mode tensor_scalar, 327ns)               ~6.7us
  Pool: p1t = iv*y2cT, p2t = q*ycT via tensor_tensor            ~4.3us
  DMA : 3 transposes/unit: xT,y2cT on the SP queue, ycT on the ACT queue

Fill: 5 junk matmuls on memset tiles warm the PE p-state ramp during the
startup DMA shadow; first transposes issue at t=0 on two queues.  Drain:
group-15 products run on DVE (fp16 tensor_tensor 2x mode) in parallel with
Pool, single output DMA.

Precision: fp16 tensors except iv (f32 - its rounding is the dominant error
term), u, PSUM, and accumulators.  yc/y2c are rounded once on the host from
f64 centering of the exact inputs; their fp16 noise is zero-mean and
incoherent across the 16.7M samples.  Measured ~4e-3 relative error.
"""

import numpy as np

N_CORES = 8
N = 131072
D = 128
X_DIM = 128
H2 = 512
M = N // N_CORES          # rows per core = 16384
RG = 1024                 # rows per group
NG = M // RG              # groups per core = 16
NEG_SLOPE = 0.2
N_WARM = 0                # junk matmuls to warm the PE p-state ramp
Q_SPLIT = 650             # q col split: piece sizes 650/374

_leaky_op = None


def _get_leaky_op():
    """Custom DVE uop: out = max((in0 + s0) * imm2, in0 + s0) — fused
    bias-add + leaky-relu in one 1x pass straight from PSUM."""
    global _leaky_op
    if _leaky_op is not None:
        return _leaky_op
    import concourse.dve_ops as DO
    from concourse.dve_spec import C0, C2, Spec, Src0, maxx

    op = DO.DveOp(
        "LEAKY_BIAS_ANT",
        Spec(
            body=maxx((Src0 + C0) * C2, Src0 + C0),
            reference=lambda in0, in1, s0, s1, imm2: np.maximum(
                (in0.astype(np.float32) + s0) * imm2,
                in0.astype(np.float32) + s0),
        ),
        subdim=False,
        uops_sha={"v3": "28ce115f5da0f06f", "v4": ""},
    )
    DO.OPS.append(op)
    DO.CUSTOM_DVE_SPECS[op.name] = op.spec
    DO._SUB_OPCODE_FOR_NAME[op.name] = DO._CUSTOM_DVE_ROW_BASE + len(DO.OPS) - 1
    assert DO._SUB_OPCODE_FOR_NAME[op.name] < 0x20
    _leaky_op = op
    return op


_compiled = None


def _build():
    import concourse.bacc as bacc
    import concourse.tile as tile
    import concourse.mybir as mybir

    F32 = mybir.dt.float32
    F16 = mybir.dt.float16
    AF = mybir.ActivationFunctionType
    OP = mybir.AluOpType

    nc = bacc.Bacc("TRN2", target_bir_lowering=False, debug=False,
                   num_devices=N_CORES)

    x_d = nc.dram_tensor("x16", [M, X_DIM], F16, kind="ExternalInput")
    yc_d = nc.dram_tensor("yc16", [M, D], F16, kind="ExternalInput")
    y2c_d = nc.dram_tensor("y2c16", [M, D], F16, kind="ExternalInput")
    # w1pack cols: [0:512) W1m, [512:1024) W1v; w2pack likewise with
    # W2'[p, c*128+d] = W2[c*128+p, d].
    w1pack_d = nc.dram_tensor("w1pack16", [128, 1024], F16, kind="ExternalInput")
    w2pack_d = nc.dram_tensor("w2pack16", [128, 1024], F16, kind="ExternalInput")
    # bpack cols: [0:4) b1m', [4:8) b1v' (b1'[p,c] = b1[c*128+p]), [8] b2m,
    # [9] -b2v
    bpack_d = nc.dram_tensor("bpack32", [128, 10], F32, kind="ExternalInput")
    # acc cols: [0:NG) P1 per group, [NG:2NG) P2 per group
    out_d = nc.dram_tensor("out", [D, 2 * NG], F32, kind="ExternalOutput")

    leaky_op = _get_leaky_op()

    with tile.TileContext(nc) as tc:
        with (
            tc.tile_pool(name="consts", bufs=1) as consts,
            tc.tile_pool(name="xtp", bufs=2) as xtp,
            tc.tile_pool(name="ytp", bufs=5) as ytp,
            tc.tile_pool(name="hidden", bufs=2) as hidden,
            tc.tile_pool(name="l2", bufs=2) as l2pool,
            tc.tile_pool(name="junk", bufs=2) as junk,
            tc.tile_pool(name="hpsum", bufs=2, space="PSUM") as hpsum,
            tc.tile_pool(name="l2psum", bufs=1, space="PSUM") as l2psum,
        ):
            # --- startup DMAs: spread across the three HWDGE queues so the
            # first xT lands ~2.4us in (per-queue issue is completion-sem
            # serialized) ---
            bp = consts.tile([128, 10], F32, tag="bp")
            nc.gpsimd.dma_start(bp[:], bpack_d[:])
            w1p = consts.tile([128, 1024], F16, tag="w1p")
            nc.scalar.dma_start(w1p[:], w1pack_d[:])
            w2p = consts.tile([128, 1024], F16, tag="w2p")
            nc.scalar.dma_start(w2p[:], w2pack_d[:])

            # PE warm-up: junk matmuls on memset tiles ride out the p-state
            # ramp inside the startup DMA shadow.
            jw = consts.tile([128, 128], F16, tag="jw")
            nc.vector.memset(jw[:], 0.0)
            jx = consts.tile([128, 512], F16, tag="jx")
            nc.vector.memset(jx[:], 0.0)
            jp = l2psum.tile([128, RG], F32, tag="zps", name="jp")
            for _ in range(N_WARM):
                nc.tensor.matmul(jp[:, :512], jw[:], jx[:],
                                 start=True, stop=True)

            # Prime the ACT function table (Prelu/Tanh/Exp set) during the
            # startup DMA shadow.
            warm = consts.tile([128, 1], F32, tag="warm")
            nc.vector.memset(warm[:], 1.0)
            nc.scalar.activation(warm[:], warm[:], AF.Exp)

            acc = consts.tile([D, 2 * NG], F32, tag="acc", name="acc")
            qacc = consts.tile([D, 2], F32, tag="qacc", name="qacc")

            def w1(k, c):
                return w1p[:, k * 512 + c * 128:k * 512 + (c + 1) * 128]

            def w2(k, c):
                return w2p[:, k * 512 + c * 128:k * 512 + (c + 1) * 128]

            def b1(k, c):
                return bp[:, k * 4 + c:k * 4 + c + 1]

            b2m = bp[:, 8:9]
            nb2v = bp[:, 9:10]

            def load_group(g):
                xT = xtp.tile([X_DIM, RG], F16, tag="xT")
                ycT = ytp.tile([D, RG], F16, tag="ycT")
                y2cT = ytp.tile([D, RG], F16, tag="y2cT")
                rows = slice(g * RG, (g + 1) * RG)
                nc.sync.dma_start_transpose(xT[:], x_d[rows, :])
                nc.sync.dma_start_transpose(y2cT[:], y2c_d[rows, :])
                nc.sync.dma_start_transpose(ycT[:], yc_d[rows, :])
                return xT, ycT, y2cT

            loads = [load_group(0), load_group(1)]
            hts_hist = {}     # g -> dict u -> ht tile
            iv_hist = {}      # g -> iv tile
            u_hist = {}       # g -> u tile
            q_hist = {}       # g -> q tile
            mups_hist = {}    # g -> mups psum tile
            p1t_hist = {}     # g -> p1t tile
            p2t_hist = {}     # g -> p2t tile

            def emit_L1_chunk(g, c, xT):
                for k in range(2):
                    hp = hpsum.tile([128, RG], F32, tag="hp")
                    for s in range(2):
                        nc.tensor.matmul(hp[:, s * 512:(s + 1) * 512],
                                         w1(k, c),
                                         xT[:, s * 512:(s + 1) * 512],
                                         start=True, stop=True)
                    ht = hidden.tile([128, RG], F16, tag=f"hT{k}{c}")
                    if k == 1:
                        nc.vector._custom_dve(
                            leaky_op, out=ht[:], in0=hp[:],
                            s0=b1(k, c), imm2=NEG_SLOPE)
                    else:
                        nc.scalar.activation(ht[:], hp[:], AF.Prelu,
                                             bias=b1(k, c),
                                             scale=1.0, alpha=NEG_SLOPE)
                    hts_hist[g][c * 2 + k] = ht

            def emit_L2_slot(slot, hts, mups, zps):
                # slots 0,1 -> z-head (k=1), slots 2,3 -> mu-head (k=0)
                k = 1 if slot < 2 else 0
                ps = zps if k == 1 else mups
                for c in ((0, 1) if slot % 2 == 0 else (2, 3)):
                    for s in range(2):
                        nc.tensor.matmul(ps[:, s * 512:(s + 1) * 512],
                                         w2(k, c),
                                         hts[c * 2 + k][:, s * 512:(s + 1) * 512],
                                         start=(c == 0), stop=(c == 3))

            def emit_tanh(g, zps):
                u = l2pool.tile([D, RG], F32, tag="u")
                nc.scalar.activation(u[:], zps[:], AF.Tanh,
                                     bias=nb2v, scale=-1.0)
                u_hist[g] = u

            def emit_exp(g):
                # iv stays f32: its fp16 rounding was the dominant error term.
                iv = l2pool.tile([D, RG], F32, tag="iv", bufs=3)
                nc.scalar.activation(iv[:], u_hist.pop(g)[:], AF.Exp)
                iv_hist[g] = iv

            def emit_q(g):
                """q = (mups + b2m) * iv in 2 col-pieces so the mups PSUM is
                freed well before this unit's slot-2 mu writes."""
                iv = iv_hist[g]
                mups = mups_hist.pop(g)
                q = l2pool.tile([D, RG], F16, tag="q")
                for j, (c0, c1) in enumerate(((0, Q_SPLIT), (Q_SPLIT, RG))):
                    nc.vector.affine_mul_reduce(
                        out=q[:, c0:c1], accum_out=qacc[:, j:j + 1],
                        in0=mups[:, c0:c1], in1=iv[:, c0:c1],
                        scale=1.0, bias=b2m)
                q_hist[g] = q

            def emit_products(g):
                """Pool: p1t = iv*y2cT, p2t = q*ycT (tensor_tensor, the only
                legal Pool elementwise op)."""
                iv, ycT, y2cT = iv_hist.pop(g), loads[g][1], loads[g][2]
                p1t = l2pool.tile([D, RG], F16, tag="p1t", bufs=3)
                nc.gpsimd.tensor_tensor(out=p1t[:], in0=iv[:], in1=y2cT[:],
                                        op=OP.mult)
                p2t = l2pool.tile([D, RG], F16, tag="p2t", bufs=3)
                nc.gpsimd.tensor_tensor(out=p2t[:], in0=q_hist.pop(g)[:],
                                        in1=ycT[:], op=OP.mult)
                p1t_hist[g] = p1t
                p2t_hist[g] = p2t

            def emit_sums(g):
                """4x-mode DVE tensor_scalar sums of p1t/p2t into acc."""
                for src, col in ((p1t_hist.pop(g), g), (p2t_hist.pop(g), NG + g)):
                    j = junk.tile([D, RG], F16, tag="jsum")
                    nc.vector.tensor_scalar(
                        out=j[:], in0=src[:], scalar1=1.0, scalar2=None,
                        op0=OP.mult, op1=OP.add,
                        accum_out=acc[:, col:col + 1])

            for g in range(NG):
                if g + 2 < NG:
                    loads.append(load_group(g + 2))
                if g >= 2:
                    emit_q(g - 2)
                if g >= 3:
                    emit_sums(g - 3)
                if g >= 2:
                    emit_products(g - 2)
                hts_hist[g] = {}
                if g >= 1:
                    mups = l2psum.tile([D, RG], F32, tag="mups")
                    zps = l2psum.tile([D, RG], F32, tag="zps")
                    mups_hist[g - 1] = mups
                for slot in range(4):
                    if g >= 1:
                        emit_L2_slot(slot, hts_hist[g - 1], mups, zps)
                        if slot == 1:
                            emit_tanh(g - 1, zps)
                        if slot == 2:
                            emit_exp(g - 1)
                    emit_L1_chunk(g, slot, loads[g][0])
                if g >= 1:
                    del hts_hist[g - 1]

            # ---- drain: unit 16 = L2(15) + tanh/exp(15) + products(14,15) ----
            g15 = NG - 1
            mups = l2psum.tile([D, RG], F32, tag="mups")
            zps = l2psum.tile([D, RG], F32, tag="zps")
            mups_hist[g15] = mups
            for slot in range(4):
                emit_L2_slot(slot, hts_hist[g15], mups, zps)
                if slot == 1:
                    emit_tanh(g15, zps)
                if slot == 2:
                    emit_exp(g15)
            emit_q(NG - 2)
            emit_sums(NG - 3)
            emit_products(NG - 2)

            # group 15 tail: q on DVE, p1t on Pool in parallel, p2t on DVE
            # (fp16 tensor_tensor runs in 2x mode), sums interleaved.
            iv15, ycT15, y2cT15 = iv_hist[g15], loads[g15][1], loads[g15][2]
            mups15 = mups_hist.pop(g15)
            q15 = l2pool.tile([D, RG], F16, tag="q")
            for j, (c0, c1) in enumerate(((0, Q_SPLIT), (Q_SPLIT, RG))):
                nc.vector.affine_mul_reduce(
                    out=q15[:, c0:c1], accum_out=qacc[:, j:j + 1],
                    in0=mups15[:, c0:c1], in1=iv15[:, c0:c1],
                    scale=1.0, bias=b2m)
            p1t15 = l2pool.tile([D, RG], F16, tag="p1t", bufs=3)
            nc.gpsimd.tensor_tensor(out=p1t15[:], in0=iv15[:], in1=y2cT15[:],
                                    op=OP.mult)
            p2t15 = l2pool.tile([D, RG], F16, tag="p2t", bufs=3)
            nc.vector.tensor_tensor(out=p2t15[:], in0=q15[:], in1=ycT15[:],
                                    op=OP.mult)
            jP2 = junk.tile([D, RG], F16, tag="jsum")
            nc.vector.tensor_scalar(
                out=jP2[:], in0=p2t15[:], scalar1=1.0, scalar2=None,
                op0=OP.mult, op1=OP.add,
                accum_out=acc[:, 2 * NG - 1:2 * NG])
            emit_sums(NG - 2)
            jP1 = junk.tile([D, RG], F16, tag="jsum")
            nc.vector.tensor_scalar(
                out=jP1[:], in0=p1t15[:], scalar1=1.0, scalar2=None,
                op0=OP.mult, op1=OP.add,
                accum_out=acc[:, NG - 1:NG])
            nc.sync.dma_start(out_d[:], acc[:])

    nc.compile()
    return nc


def _get_compiled():
    global _compiled
    if _compiled is None:
        _compiled = _build()
    return _compiled


def make_in_maps(x_samples, y_samples, W1m, b1m, W2m, b2m, W1v, b1v, W2v, b2v):
    """Host-side staging: shard x/y over cores, center y on the host, cast
    to fp16, pack weights."""
    f16 = np.float16
    f32 = np.float32
    f64 = np.float64

    def w2_shuffle(W2):
        return (np.asarray(W2, f32).reshape(4, 128, D).transpose(1, 0, 2)
                .reshape(128, 4 * D))

    w1pack = np.concatenate([
        np.asarray(W1m, f32), np.asarray(W1v, f32)], axis=1).astype(f16)
    w2pack = np.concatenate([
        w2_shuffle(W2m), w2_shuffle(W2v)], axis=1).astype(f16)
    bpack = np.concatenate([
        np.asarray(b1m, f32).reshape(4, 128).T,
        np.asarray(b1v, f32).reshape(4, 128).T,
        np.asarray(b2m, f32).reshape(128, 1),
        -np.asarray(b2v, f32).reshape(128, 1)], axis=1)
    shared = {
        "w1pack16": np.ascontiguousarray(w1pack),
        "w2pack16": np.ascontiguousarray(w2pack),
        "bpack32": np.ascontiguousarray(bpack.astype(f32)),
    }
    xs = np.asarray(x_samples, f32).astype(f16)
    y64 = np.asarray(y_samples, f32).astype(f64)
    ycs = (y64 - y64.mean(axis=0)).astype(f16)
    y2 = y64 * y64
    y2cs = (y2 - y2.mean(axis=0)).astype(f16)
    in_maps = []
    for i in range(N_CORES):
        sl = slice(i * M, (i + 1) * M)
        m = {"x16": np.ascontiguousarray(xs[sl]),
             "yc16": np.ascontiguousarray(ycs[sl]),
             "y2c16": np.ascontiguousarray(y2cs[sl])}
        m.update(shared)
        in_maps.append(m)
    return in_maps


def kernel(x_samples, y_samples, W1m, b1m, W2m, b2m, W1v, b1v, W2v, b2v):
    from concourse.bass_utils import run_bass_kernel_spmd

    nc = _get_compiled()
    in_maps = make_in_maps(x_samples, y_samples, W1m, b1m, W2m, b2m,
                           W1v, b1v, W2v, b2v)
    res = run_bass_kernel_spmd(nc, in_maps, list(range(N_CORES)))
    return combine([r["out"] for r in res.results])


def combine(outs):
    """Host-side gather: sum per-core [D, 2*NG] partials and finish the loss."""
    tot = np.sum([o.astype(np.float64) for o in outs], axis=0)
    P1 = tot[:, :NG].sum()
    P2 = tot[:, NG:].sum()
    return np.float32(-0.5 * (P1 - 2.0 * P2) / N)


# revision 6
# speedup vs baseline: 1.0165x; 1.0165x over previous
"""Trainium2 Bass kernel for nn_CLUB_816043786555 (CLUB loss).

Full-input contract: kernel(**inputs) takes the complete arrays, shards the
batch dim across 8 NeuronCores, runs a Bass/Tile kernel per core, and
combines tiny per-core partial sums on the host.

Math: with mu = leaky(x@W1m+b1m)@W2m+b2m, logvar = tanh(leaky(x@W1v+b1v)@W2v+b2v),
iv = exp(-logvar), and HOST-precomputed centered targets
  yc  = y - mean_i(y)          [per column d]
  y2c = y^2 - mean_i(y^2)
the loss reduces to (mu^2 cancels analytically):

  loss = -0.5/N * sum_{i,d} iv * (y2c - 2*mu*yc)
       = -0.5/N * (P1 - 2*P2),   P1 = sum iv*y2c,  P2 = sum q*yc,
                                 q  = (mu_psum + b2m)*iv

Host centering eliminates the B/C/S/T accumulators of the uncentered
expansion: the device emits only P1/P2 per 1024-row group (acc[128, 32]).

Schedule: 16 groups of RG=1024 rows, unit period == PE busy time (6827ns =
32 matmuls x 512 cols x 0.417ns).  KEY ORDER CHANGE vs the naive interleave:
each unit runs the L2 slots of the PREVIOUS group FIRST in each quarter
(z-head slots 0-1, mu-head slots 2-3), then the L1 chunk.  That spaces the 8
L1 PSUM writes ~1700ns apart so a plain 2-buffer hp ring gives every leaky
~3400ns of slack - no PSUM-ring stalls, no zps borrow.  tanh(g-1) releases
after slot 1 (~2.7us), exp(g-1) runs the SAME unit, so iv(g-1) is ready a
full unit before q(g-1).

Engine budget per unit (PE 6827 = pacer):
  PE  : 4 L2 slots (4 mm each) + 4 L1 chunks (4 mm each), fp16, 512-col
  ACT : 4x Prelu (bias fused) + tanh + exp                      ~6.4us
  DVE : 4x custom fused bias+leaky (PSUM) + q (scalar_tensor_tensor,
        2 col-pieces 650/374 so mups frees before slot 2 writes) +
        P1/P2 sums (4x-mode tensor_scalar, 327ns)               ~6.7us
  Pool: p1t = iv*y2cT, p2t = q*ycT via tensor_tensor            ~4.3us
  DMA : 3 transposes/unit: xT,y2cT on the SP queue, ycT on the ACT queue

Fill: 5 junk matmuls on memset tiles warm the PE p-state ramp during the
startup DMA shadow; first transposes issue at t=0 on two queues.  Drain:
group-15 products run on DVE (fp16 tensor_tensor 2x mode) in parallel with
Pool, single output DMA.

Precision: fp16 tensors except iv (f32 - its rounding is the dominant error
term), u, PSUM, and accumulators.  yc/y2c are rounded once on the host from
f64 centering of the exact inputs; their fp16 noise is zero-mean and
incoherent across the 16.7M samples.  Measured ~4e-3 relative error.
"""

import numpy as np

N_CORES = 8
N = 131072
D = 128
X_DIM = 128
H2 = 512
M = N // N_CORES          # rows per core = 16384
RG = 1024                 # rows per group
NG = M // RG              # groups per core = 16
NEG_SLOPE = 0.2
N_WARM = 5                # junk matmuls to warm the PE p-state ramp
Q_SPLIT = 650             # q col split: piece sizes 650/374

_leaky_op = None


def _get_leaky_op():
    """Custom DVE uop: out = max((in0 + s0) * imm2, in0 + s0) — fused
    bias-add + leaky-relu in one 1x pass straight from PSUM."""
    global _leaky_op
    if _leaky_op is not None:
        return _leaky_op
    import concourse.dve_ops as DO
    from concourse.dve_spec import C0, C2, Spec, Src0, maxx

    op = DO.DveOp(
        "LEAKY_BIAS_ANT",
        Spec(
            body=maxx((Src0 + C0) * C2, Src0 + C0),
            reference=lambda in0, in1, s0, s1, imm2: np.maximum(
                (in0.astype(np.float32) + s0) * imm2,
                in0.astype(np.float32) + s0),
        ),
        subdim=False,
        uops_sha={"v3": "28ce115f5da0f06f", "v4": ""},
    )
    DO.OPS.append(op)
    DO.CUSTOM_DVE_SPECS[op.name] = op.spec
    DO._SUB_OPCODE_FOR_NAME[op.name] = DO._CUSTOM_DVE_ROW_BASE + len(DO.OPS) - 1
    assert DO._SUB_OPCODE_FOR_NAME[op.name] < 0x20
    _leaky_op = op
    return op


_compiled = None


def _build():
    import concourse.bacc as bacc
    import concourse.tile as tile
    import concourse.mybir as mybir

    F32 = mybir.dt.float32
    F16 = mybir.dt.float16
    AF = mybir.ActivationFunctionType
    OP = mybir.AluOpType

    nc = bacc.Bacc("TRN2", target_bir_lowering=False, debug=False,
                   num_devices=N_CORES)

    # all three streams host-pretransposed to [feature, row-major] so the
    # per-group loads are plain contiguous DMAs (DMA-transpose is 2-byte only
    # and costs extra descriptors)
    x_d = nc.dram_tensor("xt16", [X_DIM, M], F16, kind="ExternalInput")
    yc_d = nc.dram_tensor("yct16", [D, M], F16, kind="ExternalInput")
    y2c_d = nc.dram_tensor("y2ct32", [D, M], F32, kind="ExternalInput")
    # w1pack cols: [0:512) W1m, [512:1024) W1v; w2pack likewise with
    # W2'[p, c*128+d] = W2[c*128+p, d].
    w1pack_d = nc.dram_tensor("w1pack16", [128, 1024], F16, kind="ExternalInput")
    w2pack_d = nc.dram_tensor("w2pack16", [128, 1024], F16, kind="ExternalInput")
    # bpack cols: [0:4) b1m', [4:8) b1v' (b1'[p,c] = b1[c*128+p]), [8] b2m,
    # [9] -b2v
    bpack_d = nc.dram_tensor("bpack32", [128, 10], F32, kind="ExternalInput")
    # acc cols: [0:NG) P1 per group, [NG:2NG) P2 per group
    out_d = nc.dram_tensor("out", [D, 2 * NG], F32, kind="ExternalOutput")

    leaky_op = _get_leaky_op()

    with tile.TileContext(nc) as tc:
        with (
            tc.tile_pool(name="consts", bufs=1) as consts,
            tc.tile_pool(name="xtp", bufs=2) as xtp,
            tc.tile_pool(name="ytp", bufs=5) as ytp,
            tc.tile_pool(name="hidden", bufs=2) as hidden,
            tc.tile_pool(name="l2", bufs=2) as l2pool,
            tc.tile_pool(name="junk", bufs=2) as junk,
            tc.tile_pool(name="hpsum", bufs=2, space="PSUM") as hpsum,
            tc.tile_pool(name="l2psum", bufs=1, space="PSUM") as l2psum,
        ):
            # --- startup DMAs: spread across the three HWDGE queues so the
            # first xT lands ~2.4us in (per-queue issue is completion-sem
            # serialized) ---
            bp = consts.tile([128, 10], F32, tag="bp")
            nc.gpsimd.dma_start(bp[:], bpack_d[:])
            w1p = consts.tile([128, 1024], F16, tag="w1p")
            nc.scalar.dma_start(w1p[:], w1pack_d[:])
            w2p = consts.tile([128, 1024], F16, tag="w2p")
            nc.scalar.dma_start(w2p[:], w2pack_d[:])

            # PE warm-up: junk matmuls on memset tiles ride out the p-state
            # ramp inside the startup DMA shadow.
            jw = consts.tile([128, 128], F16, tag="jw")
            nc.vector.memset(jw[:], 0.0)
            jx = consts.tile([128, 512], F16, tag="jx")
            nc.vector.memset(jx[:], 0.0)
            jp = hpsum.tile([128, RG], F32, tag="hp", name="jp")
            for _ in range(N_WARM):
                nc.tensor.matmul(jp[:, :512], jw[:], jx[:],
                                 start=True, stop=True)

            # Prime the ACT function table (Prelu/Tanh/Exp set) during the
            # startup DMA shadow.
            warm = consts.tile([128, 1], F32, tag="warm")
            nc.vector.memset(warm[:], 1.0)
            nc.scalar.activation(warm[:], warm[:], AF.Exp)

            acc = consts.tile([D, 2 * NG], F32, tag="acc", name="acc")
            qacc = consts.tile([D, 2], F32, tag="qacc", name="qacc")

            def w1(k, c):
                return w1p[:, k * 512 + c * 128:k * 512 + (c + 1) * 128]

            def w2(k, c):
                return w2p[:, k * 512 + c * 128:k * 512 + (c + 1) * 128]

            def b1(k, c):
                return bp[:, k * 4 + c:k * 4 + c + 1]

            b2m = bp[:, 8:9]
            nb2v = bp[:, 9:10]

            def load_group(g):
                xT = xtp.tile([X_DIM, RG], F16, tag="xT")
                ycT = ytp.tile([D, RG], F16, tag="ycT")
                y2cT = ytp.tile([D, RG], F32, tag="y2cT")
                rows = slice(g * RG, (g + 1) * RG)
                nc.sync.dma_start(xT[:], x_d[:, rows])
                nc.sync.dma_start(y2cT[:], y2c_d[:, rows])
                nc.scalar.dma_start(ycT[:], yc_d[:, rows])
                return xT, ycT, y2cT

            loads = [load_group(0), load_group(1)]
            hts_hist = {}     # g -> dict u -> ht tile
            iv_hist = {}      # g -> iv tile
            u_hist = {}       # g -> u tile
            q_hist = {}       # g -> q tile
            mups_hist = {}    # g -> mups psum tile
            p1t_hist = {}     # g -> p1t tile
            p2t_hist = {}     # g -> p2t tile

            def emit_L1_chunk(g, c, xT):
                for k in range(2):
                    hp = hpsum.tile([128, RG], F32, tag="hp")
                    for s in range(2):
                        nc.tensor.matmul(hp[:, s * 512:(s + 1) * 512],
                                         w1(k, c),
                                         xT[:, s * 512:(s + 1) * 512],
                                         start=True, stop=True)
                    ht = hidden.tile([128, RG], F16, tag=f"hT{k}{c}")
                    if k == 1:
                        nc.vector._custom_dve(
                            leaky_op, out=ht[:], in0=hp[:],
                            s0=b1(k, c), imm2=NEG_SLOPE)
                    else:
                        nc.scalar.activation(ht[:], hp[:], AF.Prelu,
                                             bias=b1(k, c),
                                             scale=1.0, alpha=NEG_SLOPE)
                    hts_hist[g][c * 2 + k] = ht

            def emit_L2_slot(slot, hts, mups, zps):
                # slots 0,1 -> z-head (k=1), slots 2,3 -> mu-head (k=0)
                k = 1 if slot < 2 else 0
                ps = zps if k == 1 else mups
                for c in ((0, 1) if slot % 2 == 0 else (2, 3)):
                    for s in range(2):
                        nc.tensor.matmul(ps[:, s * 512:(s + 1) * 512],
                                         w2(k, c),
                                         hts[c * 2 + k][:, s * 512:(s + 1) * 512],
                                         start=(c == 0), stop=(c == 3))

            def emit_tanh(g, zps):
                u = l2pool.tile([D, RG], F32, tag="u")
                nc.scalar.activation(u[:], zps[:], AF.Tanh,
                                     bias=nb2v, scale=-1.0)
                u_hist[g] = u

            def emit_exp(g):
                # iv stays f32: its fp16 rounding was the dominant error term.
                iv = l2pool.tile([D, RG], F32, tag="iv", bufs=3)
                nc.scalar.activation(iv[:], u_hist.pop(g)[:], AF.Exp)
                iv_hist[g] = iv

            def emit_q(g):
                """q = (mups + b2m) * iv in 2 col-pieces so the mups PSUM is
                freed well before this unit's slot-2 mu writes."""
                iv = iv_hist[g]
                mups = mups_hist.pop(g)
                q = l2pool.tile([D, RG], F16, tag="q")
                for j, (c0, c1) in enumerate(((0, Q_SPLIT), (Q_SPLIT, RG))):
                    nc.vector.affine_mul_reduce(
                        out=q[:, c0:c1], accum_out=qacc[:, j:j + 1],
                        in0=mups[:, c0:c1], in1=iv[:, c0:c1],
                        scale=1.0, bias=b2m)
                q_hist[g] = q

            def emit_products(g):
                """Pool: p1t = iv*y2cT, p2t = q*ycT (tensor_tensor, the only
                legal Pool elementwise op)."""
                iv, ycT, y2cT = iv_hist.pop(g), loads[g][1], loads[g][2]
                p1t = l2pool.tile([D, RG], F16, tag="p1t", bufs=3)
                nc.gpsimd.tensor_tensor(out=p1t[:], in0=iv[:], in1=y2cT[:],
                                        op=OP.mult)
                p2t = l2pool.tile([D, RG], F16, tag="p2t", bufs=3)
                nc.gpsimd.tensor_tensor(out=p2t[:], in0=q_hist.pop(g)[:],
                                        in1=ycT[:], op=OP.mult)
                p1t_hist[g] = p1t
                p2t_hist[g] = p2t

            def emit_sums(g):
                """4x-mode DVE tensor_scalar sums of p1t/p2t into acc."""
                for src, col in ((p1t_hist.pop(g), g), (p2t_hist.pop(g), NG + g)):
                    j = junk.tile([D, RG], F16, tag="jsum")
                    nc.vector.tensor_scalar(
                        out=j[:], in0=src[:], scalar1=1.0, scalar2=None,
                        op0=OP.mult, op1=OP.add,
                        accum_out=acc[:, col:col + 1])

            for g in range(NG):
                if g + 2 < NG:
                    loads.append(load_group(g + 2))
                if g >= 2:
                    emit_q(g - 2)
                if g >= 3:
                    emit_sums(g - 3)
                if g >= 2:
                    emit_products(g - 2)
                hts_hist[g] = {}
                if g >= 1:
                    mups = l2psum.tile([D, RG], F32, tag="mups")
                    zps = l2psum.tile([D, RG], F32, tag="zps")
                    mups_hist[g - 1] = mups
                for slot in range(4):
                    if g >= 1:
                        emit_L2_slot(slot, hts_hist[g - 1], mups, zps)
                        if slot == 1:
                            emit_tanh(g - 1, zps)
                        if slot == 2:
                            emit_exp(g - 1)
                    emit_L1_chunk(g, slot, loads[g][0])
                if g >= 1:
                    del hts_hist[g - 1]

            # ---- drain: unit 16 = L2(15) + tanh/exp(15) + products(14,15) ----
            g15 = NG - 1
            mups = l2psum.tile([D, RG], F32, tag="mups")
            zps = l2psum.tile([D, RG], F32, tag="zps")
            mups_hist[g15] = mups
            for slot in range(4):
                emit_L2_slot(slot, hts_hist[g15], mups, zps)
                if slot == 1:
                    emit_tanh(g15, zps)
                if slot == 2:
                    emit_exp(g15)
            emit_q(NG - 2)
            emit_sums(NG - 3)
            emit_products(NG - 2)

            # group 15 tail: q on DVE, p1t on Pool in parallel, p2t on DVE
            # (fp16 tensor_tensor runs in 2x mode), sums interleaved.
            iv15, ycT15, y2cT15 = iv_hist[g15], loads[g15][1], loads[g15][2]
            mups15 = mups_hist.pop(g15)
            q15 = l2pool.tile([D, RG], F16, tag="q")
            for j, (c0, c1) in enumerate(((0, Q_SPLIT), (Q_SPLIT, RG))):
                nc.vector.affine_mul_reduce(
                    out=q15[:, c0:c1], accum_out=qacc[:, j:j + 1],
                    in0=mups15[:, c0:c1], in1=iv15[:, c0:c1],
                    scale=1.0, bias=b2m)
            p1t15 = l2pool.tile([D, RG], F16, tag="p1t", bufs=3)
            nc.gpsimd.tensor_tensor(out=p1t15[:], in0=iv15[:], in1=y2cT15[:],
                                    op=OP.mult)
            p2t15 = l2pool.tile([D, RG], F16, tag="p2t", bufs=3)
            nc.vector.tensor_tensor(out=p2t15[:], in0=q15[:], in1=ycT15[:],
                                    op=OP.mult)
            jP2 = junk.tile([D, RG], F16, tag="jsum")
            nc.vector.tensor_scalar(
                out=jP2[:], in0=p2t15[:], scalar1=1.0, scalar2=None,
                op0=OP.mult, op1=OP.add,
                accum_out=acc[:, 2 * NG - 1:2 * NG])
            emit_sums(NG - 2)
            jP1 = junk.tile([D, RG], F16, tag="jsum")
            nc.vector.tensor_scalar(
                out=jP1[:], in0=p1t15[:], scalar1=1.0, scalar2=None,
                op0=OP.mult, op1=OP.add,
                accum_out=acc[:, NG - 1:NG])
            nc.sync.dma_start(out_d[:], acc[:])

    nc.compile()
    return nc


def _get_compiled():
    global _compiled
    if _compiled is None:
        _compiled = _build()
    return _compiled


def make_in_maps(x_samples, y_samples, W1m, b1m, W2m, b2m, W1v, b1v, W2v, b2v):
    """Host-side staging: shard x/y over cores, center y on the host, cast
    to fp16, pack weights."""
    f16 = np.float16
    f32 = np.float32
    f64 = np.float64

    def w2_shuffle(W2):
        return (np.asarray(W2, f32).reshape(4, 128, D).transpose(1, 0, 2)
                .reshape(128, 4 * D))

    w1pack = np.concatenate([
        np.asarray(W1m, f32), np.asarray(W1v, f32)], axis=1).astype(f16)
    w2pack = np.concatenate([
        w2_shuffle(W2m), w2_shuffle(W2v)], axis=1).astype(f16)
    bpack = np.concatenate([
        np.asarray(b1m, f32).reshape(4, 128).T,
        np.asarray(b1v, f32).reshape(4, 128).T,
        np.asarray(b2m, f32).reshape(128, 1),
        -np.asarray(b2v, f32).reshape(128, 1)], axis=1)
    shared = {
        "w1pack16": np.ascontiguousarray(w1pack),
        "w2pack16": np.ascontiguousarray(w2pack),
        "bpack32": np.ascontiguousarray(bpack.astype(f32)),
    }
    xs = np.asarray(x_samples, f32).astype(f16)
    y64 = np.asarray(y_samples, f32).astype(f64)
    ycs = (y64 - y64.mean(axis=0)).astype(f16)
    y2 = y64 * y64
    y2cs = (y2 - y2.mean(axis=0)).astype(f32)
    in_maps = []
    for i in range(N_CORES):
        sl = slice(i * M, (i + 1) * M)
        m = {"xt16": np.ascontiguousarray(xs[sl].T),
             "yct16": np.ascontiguousarray(ycs[sl].T),
             "y2ct32": np.ascontiguousarray(y2cs[sl].T)}
        m.update(shared)
        in_maps.append(m)
    return in_maps


def kernel(x_samples, y_samples, W1m, b1m, W2m, b2m, W1v, b1v, W2v, b2v):
    from concourse.bass_utils import run_bass_kernel_spmd

    nc = _get_compiled()
    in_maps = make_in_maps(x_samples, y_samples, W1m, b1m, W2m, b2m,
                           W1v, b1v, W2v, b2v)
    res = run_bass_kernel_spmd(nc, in_maps, list(range(N_CORES)))
    return combine([r["out"] for r in res.results])


def combine(outs):
    """Host-side gather: sum per-core [D, 2*NG] partials and finish the loss."""
    tot = np.sum([o.astype(np.float64) for o in outs], axis=0)
    P1 = tot[:, :NG].sum()
    P2 = tot[:, NG:].sum()
    return np.float32(-0.5 * (P1 - 2.0 * P2) / N)
